# revision 3
# baseline (speedup 1.0000x reference)
"""Trainium2 Bass kernel for nn_CNN3_FPB (dense CNN + bypass MLP + FC head).

Data-parallel over 8 NeuronCores: batch 2048 -> 256 per core. All weights
replicated. Inside each core:

  warmup: dummy matmuls on the framework's const AP (ready at the preamble
          barrier, no DMA/memset deps) bridge the initial x-DMA wait and
          get the PE HAM clock-gate to 8/8 before real work starts.
  stage0: y[p,b,c] = Wp1 @ xT[:,b,c] + Wp0 @ x0[:,b] + bp, relu
          (K=128 matmul, M=64 -> column-tiled: two concurrent M=64 matmuls
          in array col-halves). The epilogue dual-writes each psum chunk
          into BOTH h1d halves (rows 64:128 get the shift-by-one copy via
          a partition-shifted ACT write) -- no SBUF->SBUF copy DMA.
  conv1:  per chunk: taps(0,1) as one K=128 matmul on h1d; the K=64 tap2
          matmuls of ADJACENT chunks go on opposite PE row-group halves
          (top reads y[p+1] at h1d-top col p+2, bottom at h1d-bot col p+1)
          so they run concurrently, each into its own chunk's psum.
  conv2:  K=3 stride 2, Cin=128, Cout=256 (2 M-chunks), 3 K=128 matmuls;
          taps read h2 via stride-2 column APs.
  conv3:  K=3 stride 2, Cin=256 (2 K-chunks), Cout=256 (2 M-chunks), 6
          accumulating K=128 matmuls, stride-2 tap reads.

  h2/h3 are stored in NATURAL position order (col c = out pos c-1, padded
  ends) so every conv epilogue chunk is ONE contiguous ACT/DVE op -- the
  scalar engine was the secondary bottleneck with parity-split layouts.

  The block loop is software-pipelined: iteration b emits
      stage0(b), conv3(b-2), conv2(b-1), conv1(b)
  so conv2(b-1) sees ~3us of PE work in front of its dependency on
  conv1(b-1)'s epilogue and conv1(b) sees ~7us in front of stage0(b)'s
  (PE stalls also re-throttle the HAM clock to 1.2GHz).

  fc1:    f-outer: per 128-wide f-chunk, stream its weights in 8 chunks
          (round-robin DMA queues, first 12 prefetched during the trunk),
          run 130 accumulating K=128 matmuls into one PSUM bank, relu, and
          accumulate fc2 inline in a second bank. No serial tail.

Startup DMAs are ordered small-first (x0/bypass/bias/conv weights before
the 1MB x blocks) so the bypass MLP and stage0(0) unblock early. All
matmuls bf16: fp8 was measured (hardware + faithful CPU study) at 4-7%
max-rel output error on this net in every subset -- out of tolerance
(2e-2); e4m3's ~3%/layer noise is scale-invariant and irreducible.
"""

import os
import sys
from contextlib import ExitStack

import numpy as np

for _p in ("/opt/trn_rl_repo", "/root/.axon_site/_ro/trn_rl_repo"):
    if os.path.isdir(_p) and _p not in sys.path:
        sys.path.insert(0, _p)

import ml_dtypes  # noqa: E402
import concourse.bass as bass  # noqa: E402
from concourse import bacc  # noqa: E402
import concourse.mybir as mybir  # noqa: E402
import concourse.tile as tile  # noqa: E402

F32 = mybir.dt.float32
F32R = mybir.dt.float32r
BF16 = mybir.dt.bfloat16
RELU = mybir.ActivationFunctionType.Relu
ADD = mybir.AluOpType.add
MAX = mybir.AluOpType.max

# Problem constants (hardcoded; must match the grading problem).
B, CL, IL = 2048, 256, 64
NCORES = 8
BC = B // NCORES  # 256 samples per core
BB = 16           # samples per conv block
NBLK = BC // BB
PC = 64
CH1, CH2, CH3 = 128, 256, 256
L1, L2, L3 = 255, 128, 64
F1 = 1024
OUTC = 2

NBIAS = 19   # bias columns: see _prep_bias
NWARM = 26   # dummy warmup matmuls (N=256 each) before first real work
NWARM2 = 6   # filler matmuls between stage0(0) and conv1(0)

# fc1 weight streaming: 8 chunks per f-chunk (65 slabs = 64 l3 + bypass)
NQ = 8
QS = 9
NSQ = [9, 8, 8, 8, 8, 8, 8, 8]
OFF = [0, 9, 17, 25, 33, 41, 49, 57]
NGATE = 12  # chunks prefetched during the trunk (gated, scalar queue)


def build_nc():
    nc = bacc.Bacc()
    TDT = BF16

    xs = nc.declare_dram_parameter("xs", [NBLK, 128, CL, BB], TDT, isOutput=False)
    x0s = nc.declare_dram_parameter("x0s", [64, BC], F32R, isOutput=False)
    wstk = nc.declare_dram_parameter("wstk", [128, 64], TDT, isOutput=False)
    # w1p: [:, 0, :] = [W1tap0.T ; W1tap1.T] stacked (K=128 MM1);
    # [:, 1, :] = W1tap2.T duplicated in both row halves (K=64 row-tiled MM2)
    w1p = nc.declare_dram_parameter("w1p", [128, 2, 128], TDT, isOutput=False)
    w2 = nc.declare_dram_parameter("w2", [128, 3, CH2], TDT, isOutput=False)
    w3 = nc.declare_dram_parameter("w3", [128, 2, 3, CH3], TDT, isOutput=False)
    wb1 = nc.declare_dram_parameter("wb1", [64, 64], F32R, isOutput=False)
    wb2 = nc.declare_dram_parameter("wb2", [64, 128], F32R, isOutput=False)
    wb3 = nc.declare_dram_parameter("wb3", [128, 256], F32R, isOutput=False)
    # fc1 weights, f-chunk-major chunks:
    # [f, chunk, 128 part, <=9 slabs, 2 ci, 128 fcols]
    # slab g = OFF[chunk]+s: g<64 -> l3 position g, g==64 -> bypass
    wgf = nc.declare_dram_parameter("wgf", [8, NQ, 128, QS, 2, 128], BF16,
                                    isOutput=False)
    wfc2 = nc.declare_dram_parameter("wfc2", [128, 8, OUTC], F32R, isOutput=False)
    bias = nc.declare_dram_parameter("bias", [128, NBIAS], F32, isOutput=False)
    out = nc.declare_dram_parameter("out", [OUTC, BC], F32, isOutput=True)

    with ExitStack() as ctx:
        tc = ctx.enter_context(tile.TileContext(nc))
        wpool = ctx.enter_context(tc.tile_pool(name="wpool", bufs=1))
        xpool = ctx.enter_context(tc.tile_pool(name="xpool", bufs=2))
        wgpool = ctx.enter_context(tc.tile_pool(name="wgpool", bufs=NGATE))
        h1pool = ctx.enter_context(tc.tile_pool(name="h1pool", bufs=2))
        h2pool = ctx.enter_context(tc.tile_pool(name="h2pool", bufs=2))
        h3pool = ctx.enter_context(tc.tile_pool(name="h3pool", bufs=2))
        zpool = ctx.enter_context(tc.tile_pool(name="zpool", bufs=1))
        z2pool = ctx.enter_context(tc.tile_pool(name="z2pool", bufs=2))
        spool = ctx.enter_context(tc.tile_pool(name="spool", bufs=1))

        # ---- warmup operand: the framework's const AP (memset in the
        # preamble) -- zero extra deps, so warmup MMs start right at the
        # preamble barrier (~6.5us) ----
        one_bf = nc.const_aps.aps[(BF16, 1.0)]
        one_bcast = one_bf.broadcast_to([128, 256])

        # ---- startup DMAs: small tensors first (x0/bypass/bias/conv wts)
        # so the bypass MLP and stage0(0) unblock early; the 1MB x blocks
        # follow, 3-way split across the queues ----
        xt_pre = {}
        x0_t = wpool.tile([64, BC], F32R)
        nc.sync.dma_start(x0_t[:], x0s[:])
        wstk_t = wpool.tile([128, 64], TDT)
        nc.scalar.dma_start(wstk_t[:], wstk[:])
        bias_t = wpool.tile([128, NBIAS], F32)
        nc.scalar.dma_start(bias_t[:], bias[:])
        wb1_t = wpool.tile([64, 64], F32R)
        nc.sync.dma_start(wb1_t[:], wb1[:])
        wb2_t = wpool.tile([64, 128], F32R)
        nc.sync.dma_start(wb2_t[:], wb2[:])
        w1p_t = wpool.tile([128, 2, 128], TDT)
        nc.gpsimd.dma_start(w1p_t[:], w1p[:])
        wb3_t = wpool.tile([128, 256], F32R)
        nc.sync.dma_start(wb3_t[:], wb3[:])
        w2_t = wpool.tile([128, 3, CH2], TDT)
        nc.gpsimd.dma_start(w2_t[:], w2[:])
        xt0 = xpool.tile([128, CL, BB], TDT, name="xt0", tag="xt")
        nc.sync.dma_start(xt0[:, 0:86, :], xs[0, :, 0:86, :])
        nc.scalar.dma_start(xt0[:, 86:171, :], xs[0, :, 86:171, :])
        nc.gpsimd.dma_start(xt0[:, 171:256, :], xs[0, :, 171:256, :])
        xt_pre[0] = xt0
        xt1 = xpool.tile([128, CL, BB], TDT, name="xt1", tag="xt")
        nc.sync.dma_start(xt1[:, 0:86, :], xs[1, :, 0:86, :])
        nc.scalar.dma_start(xt1[:, 86:171, :], xs[1, :, 86:171, :])
        nc.gpsimd.dma_start(xt1[:, 171:256, :], xs[1, :, 171:256, :])
        xt_pre[1] = xt1
        w3_t = wpool.tile([128, 2, 3, CH3], TDT)
        nc.scalar.dma_start(w3_t[:], w3[:])
        xt2 = xpool.tile([128, CL, BB], TDT, name="xt2", tag="xt")
        nc.sync.dma_start(xt2[:, 0:86, :], xs[2, :, 0:86, :])
        nc.gpsimd.dma_start(xt2[:, 86:171, :], xs[2, :, 86:171, :])
        nc.scalar.dma_start(xt2[:, 171:256, :], xs[2, :, 171:256, :])
        xt_pre[2] = xt2
        wfc2_t = wpool.tile([128, 8, OUTC], F32R)
        nc.gpsimd.dma_start(wfc2_t[:], wfc2[:])

        bp_lo = bias_t[:64, 0:1]
        bp_hi = bias_t[64:128, 0:1]
        b1_ap = bias_t[:, 1:2]

        cpsum_ctx = ExitStack()
        cpsum = cpsum_ctx.enter_context(tc.tile_pool(name="cpsum", bufs=8, space="PSUM"))

        # ---- warmup: one long accumulation chain of zero matmuls ----
        wps = cpsum.tile([128, 512], F32, tag="ps")
        for i in range(NWARM):
            nc.tensor.matmul(
                wps[0:1, 0:256], one_bf, one_bcast,
                start=(i == 0), stop=(i == NWARM - 1),
            )

        def bypass_mlp():
            # tiny MLP on x0; emitted after stage0(0) so it fills the PE
            # while block 0's h1d copy completes
            ps = cpsum.tile([64, BC], F32, tag="ps")
            nc.tensor.matmul(ps[:], wb1_t[:], x0_t[:], start=True, stop=True)
            s1 = spool.tile([64, BC], F32R)
            nc.scalar.activation(s1[:], ps[:], RELU, bias=bias_t[:64, 6:7])
            ps = cpsum.tile([128, BC], F32, tag="ps")
            nc.tensor.matmul(ps[:], wb2_t[:], s1[:], start=True, stop=True)
            s2 = spool.tile([128, BC], F32R)
            nc.scalar.activation(s2[:], ps[:], RELU, bias=bias_t[:, 7:8])
            fbyp = spool.tile([128, 2, BC], BF16)
            for m in range(2):
                ps = cpsum.tile([128, BC], F32, tag="ps")
                nc.tensor.matmul(
                    ps[:], wb3_t[:, m * 128 : (m + 1) * 128], s2[:],
                    start=True, stop=True,
                )
                nc.vector.tensor_scalar(
                    fbyp[:, m, :], ps[:], bias_t[:, 8 + m : 9 + m], 0.0, ADD, MAX
                )
            return fbyp

        # ---- resident conv3 output (fc1 rhs), bf16: [ci, cich, l3, b] ----
        zres = zpool.tile([128, 2, L3, BC], BF16)

        S0_CHUNKS = [(1 + 32 * j, 32 if j < 7 else 31) for j in range(8)]
        C1_CHUNKS = [(32 * j, 32 if j < 7 else 31) for j in range(8)]

        def stage0(blk):
            if blk in xt_pre:
                xt = xt_pre[blk]
            else:
                xt = xpool.tile([128, CL, BB], TDT, name="xt", tag="xt")
                nc.sync.dma_start(xt[:, 0:128, :], xs[blk, :, 0:128, :])
                nc.gpsimd.dma_start(xt[:, 128:256, :], xs[blk, :, 128:256, :])

            # stage0 -> h1d [128, 256, BB]:
            #   rows 0:64,  col j = y[pos j-1]  (j=1..255; j=0 zero pad)
            #   rows 64:128 col j = y[pos j]    (j=0..254; j=255 zero pad)
            # Both halves are written straight from PSUM (the rows-64:128
            # copy is a partition-shifted second epilogue write, not a DMA).
            h1d = h1pool.tile([128, 256, BB], TDT)
            nc.gpsimd.memset(h1d[0:64, 0:1, :], 0.0)
            nc.gpsimd.memset(h1d[64:128, 255:256, :], 0.0)
            for q in range(4):
                ps = cpsum.tile([128, 512], F32, tag="ps")
                for half in range(2):
                    c0, cc = S0_CHUNKS[2 * q + half]
                    nc.tensor.matmul(
                        ps[64 * half : 64 * half + 64, 0 : cc * BB],
                        wstk_t[:],
                        xt[:, c0 : c0 + cc, :].rearrange("p c b -> p (c b)"),
                        start=True, stop=True,
                    )
                for half in range(2):
                    c0, cc = S0_CHUNKS[2 * q + half]
                    src = ps[64 * half : 64 * half + 64, 0 : cc * BB]
                    bsl = bp_hi if half else bp_lo
                    # A: top half, col j = y[j-1] -> dst cols c0..c0+cc-1
                    dstA = h1d[0:64, c0 : c0 + cc, :].rearrange(
                        "p c b -> p (c b)")
                    # B: bottom half, col j = y[j] -> dst cols c0-1..
                    dstB = h1d[64:128, c0 - 1 : c0 - 1 + cc, :].rearrange(
                        "p c b -> p (c b)")
                    nc.vector.tensor_scalar(dstA, src, bsl, 0.0, ADD, MAX)
                    nc.scalar.activation(dstB, src, RELU, bias=bsl)
            return h1d

        def conv1(h1d):
            # conv1 -> h2n NATURAL layout [128, 258, BB]: col c = out pos c-1
            # (col 0 = pad for pos -1, col 256 = pad for pos 255, col 257
            # unused). One epilogue op per chunk -- no parity split; conv2
            # reads taps via stride-2 APs instead.
            h2n = h2pool.tile([128, 258, BB], TDT)
            nc.gpsimd.memset(h2n[:, 0:1, :], 0.0)
            nc.gpsimd.memset(h2n[:, 256:257, :], 0.0)
            # conv1 with row-tiled tap2: per chunk, MM1 packs taps (0,1) as
            # one K=128 matmul (as the baseline). The K=64 tap2 matmuls of
            # ADJACENT chunks are placed on opposite PE row-group halves
            # (top half reads y[p+1] at h1d-top col p+2, bottom half at
            # h1d-bottom col p+1) so the two run CONCURRENTLY (measured
            # 116ns/MM vs 224 serial), each accumulating into its own
            # chunk's psum. 12 PE slots/block instead of 16.
            for cp in range(0, 8, 2):
                (l0a, lca), (l0b, lcb) = C1_CHUNKS[cp], C1_CHUNKS[cp + 1]
                na, nb = lca * BB, lcb * BB
                psa = cpsum.tile([128, 512], F32, tag="ps")
                psb = cpsum.tile([128, 512], F32, tag="ps")
                nc.tensor.matmul(
                    psa[:, 0:na], w1p_t[:, 0, :],
                    h1d[:, l0a : l0a + lca, :].rearrange("p l b -> p (l b)"),
                    start=True, stop=False,
                )
                nc.tensor.matmul(
                    psb[:, 0:nb], w1p_t[:, 0, :],
                    h1d[:, l0b : l0b + lcb, :].rearrange("p l b -> p (l b)"),
                    start=True, stop=False,
                )
                nc.tensor.matmul(
                    psa[:, 0:na], w1p_t[0:64, 1, :],
                    h1d[0:64, l0a + 2 : l0a + 2 + lca, :].rearrange(
                        "p l b -> p (l b)"),
                    start=False, stop=True,
                )
                nc.tensor.matmul(
                    psb[:, 0:nb], w1p_t[64:128, 1, :],
                    h1d[64:128, l0b + 1 : l0b + 1 + lcb, :].rearrange(
                        "p l b -> p (l b)"),
                    start=False, stop=True,
                )
                for ci, (l0, lc), ps, n in ((cp, (l0a, lca), psa, na),
                                            (cp + 1, (l0b, lcb), psb, nb)):
                    dst = h2n[:, l0 + 1 : l0 + 1 + lc, :].rearrange(
                        "p l b -> p (l b)")
                    if ci % 2 == 0:
                        nc.vector.tensor_scalar(
                            dst, ps[:, 0:n], b1_ap, 0.0, ADD, MAX)
                    else:
                        nc.scalar.activation(
                            dst, ps[:, 0:n], RELU, bias=b1_ap)
            return h2n

        def conv2(h2n):
            # conv2 -> h3n NATURAL [128, 2, 130, BB]: col c = out pos c-1
            # (col 0 pad, cols 1..128 = pos 0..127, col 129 unused)
            h3n = h3pool.tile([128, 2, 130, BB], TDT)
            nc.gpsimd.memset(h3n[:, :, 0:1, :], 0.0)
            h2v = h2n[:, :, :].rearrange("p (m r) b -> p m r b", r=2)
            for m in range(2):
                for pair in range(2):
                    for i in range(2):
                        ps = cpsum.tile([128, 512], F32, tag="ps")
                        q0 = 64 * pair + 32 * i
                        # tap k reads h2n col 2q+k = h2v[m=q+dm, r=rk]
                        for k, (dm, rk) in enumerate(((0, 0), (0, 1), (1, 0))):
                            nc.tensor.matmul(
                                ps[:],
                                w2_t[:, k, m * 128 : (m + 1) * 128],
                                h2v[:, q0 + dm : q0 + dm + 32, rk, :],
                                start=(k == 0), stop=(k == 2),
                            )
                        dst = h3n[:, m, q0 + 1 : q0 + 33, :].rearrange(
                            "p l b -> p (l b)")
                        if (pair + i) % 2 == 0:
                            nc.scalar.activation(
                                dst, ps[:], RELU, bias=bias_t[:, 2 + m : 3 + m])
                        else:
                            nc.vector.tensor_scalar(
                                dst, ps[:], bias_t[:, 2 + m : 3 + m], 0.0,
                                ADD, MAX)
            return h3n

        def conv3(h3n, blk):
            b0 = blk * BB
            h3v = h3n[:, :, :, :].rearrange("p c (m r) b -> p c m r b", r=2)
            for m in range(2):
                for q in range(2):
                    ps = cpsum.tile([128, 512], F32, tag="ps")
                    l30 = 32 * q
                    acc = 0
                    for c in range(2):
                        for k, (dm, rk) in enumerate(((0, 0), (0, 1), (1, 0))):
                            nc.tensor.matmul(
                                ps[:],
                                w3_t[:, c, k, m * 128 : (m + 1) * 128],
                                h3v[:, c, l30 + dm : l30 + dm + 32, rk, :],
                                start=(acc == 0), stop=(acc == 5),
                            )
                            acc += 1
                    ps3 = ps.rearrange("p (l b) -> p l b", b=BB)
                    dst = zres[:, m, l30 : l30 + 32, b0 : b0 + BB]
                    if (m + q) % 2 == 0:
                        nc.scalar.activation(
                            dst, ps3[:], RELU, bias=bias_t[:, 4 + m : 5 + m],
                        )
                    else:
                        nc.vector.tensor_scalar(
                            dst, ps3[:], bias_t[:, 4 + m : 5 + m], 0.0, ADD, MAX,
                        )

        # ---- software-pipelined trunk ----
        # fc1 weight chunks j=0..NGATE-1 prefetch during the trunk: a
        # 1-element vector "gate" write into the tile pins each DMA's start
        # to real-time block pace (the scheduler cannot hoist it into the
        # startup window); transfers go on the scalar queue, which carries
        # no other DMAs mid-trunk (gpsimd carries the h1d copies).
        # iteration b emits: stage0(b), conv3(b-2), conv2(b-1), conv1(b):
        # conv2(b-1) gets ~3us of PE work (stage0+conv3) in front of its
        # dependency on conv1(b-1)'s epilogue; conv1(b) gets ~7us in front
        # of stage0(b)'s epilogue.
        fbyp = bypass_mlp()
        wg_tiles = {}
        prev = None   # (h2n, blk) of b-1
        prev3 = None  # (h3n, blk) of b-2
        for blk in range(NBLK):
            h1d = stage0(blk)
            if blk == 0:
                wps2 = cpsum.tile([128, 512], F32, tag="ps")
                for i in range(NWARM2):
                    nc.tensor.matmul(
                        wps2[0:1, 0:256], one_bf, one_bcast,
                        start=(i == 0), stop=(i == NWARM2 - 1),
                    )
            if prev3 is not None:
                conv3(*prev3)
                prev3 = None
            if prev is not None:
                h2p, pblk = prev
                h3n = conv2(h2p)
                prev3 = (h3n, pblk)
            if 4 <= blk < 4 + NGATE:
                j = blk - 4
                ns = NSQ[j % NQ]
                wt = wgpool.tile([128, QS, 2, 128], BF16, name="wq", tag="wgf")
                nc.vector.tensor_scalar_add(
                    wt[0:1, 0, 0, 0:1], bias_t[0:1, 0:1], 0.0
                )
                nc.scalar.dma_start(
                    wt[:, :ns, :, :], wgf[j // NQ, j % NQ, :, :ns, :, :]
                )
                wg_tiles[j] = wt
            h2n = conv1(h1d)
            prev = (h2n, blk)
        if prev3 is not None:
            conv3(*prev3)
        h2p, pblk = prev
        h3n = conv2(h2p)
        conv3(h3n, pblk)

        # ---- fc1 (f-outer) + inline fc2 ----
        cpsum_ctx.close()
        fpsum_ctx = ExitStack()
        fpsum = fpsum_ctx.enter_context(tc.tile_pool(name="fpsum", bufs=2, space="PSUM"))
        f2psum = fpsum_ctx.enter_context(tc.tile_pool(name="f2psum", bufs=1, space="PSUM"))
        f2ps = f2psum.tile([2, BC], F32, tag="ps2", name="ps2")
        for f in range(8):
            fps = fpsum.tile([128, BC], F32, tag="fps", name=f"fps{f}")
            nmm = 0
            for q in range(NQ):
                j = NQ * f + q
                ns = NSQ[q]
                if j in wg_tiles:
                    wt = wg_tiles.pop(j)
                else:
                    # slot reuse (WAR on chunk j-NGATE's matmul reads) paces
                    # these transfers; round-robin all 3 DMA queues (each
                    # sustains ~105GB/s; fc1 consumes ~300GB/s)
                    wt = wgpool.tile([128, QS, 2, 128], BF16, name="wq", tag="wgf")
                    eng = (nc.sync, nc.scalar, nc.gpsimd)[j % 3]
                    eng.dma_start(
                        wt[:, :ns, :, :], wgf[f, q, :, :ns, :, :]
                    )
                for s in range(ns):
                    g = OFF[q] + s
                    for c in range(2):
                        rhs = zres[:, c, g, :] if g < 64 else fbyp[:, c, :]
                        nc.tensor.matmul(
                            fps[:], wt[:, s, c, :], rhs,
                            start=(nmm == 0), stop=(nmm == 129),
                        )
                        nmm += 1
            z2f = z2pool.tile([128, BC], F32R, tag="z2")
            nc.scalar.activation(
                z2f[:], fps[:], RELU, bias=bias_t[:, 10 + f : 11 + f],
            )
            nc.tensor.matmul(
                f2ps[:], wfc2_t[:, f, :], z2f[:],
                start=(f == 0), stop=(f == 7),
            )

        osb = spool.tile([2, BC], F32)
        nc.vector.tensor_scalar_add(osb[:], f2ps[:], bias_t[:2, 18:19])
        nc.sync.dma_start(out[:], osb[:])
        fpsum_ctx.close()

    nc.compile()
    return nc


def _prep_inputs(inputs):
    """Host-side layout prep. Returns per-core input maps."""
    f32 = lambda a: np.ascontiguousarray(np.asarray(a), dtype=np.float32)
    x = f32(inputs["x"])
    Wp = f32(inputs["Wp"])
    W1, W2, W3 = f32(inputs["W1"]), f32(inputs["W2"]), f32(inputs["W3"])
    Wb1, Wb2, Wb3 = f32(inputs["Wb1"]), f32(inputs["Wb2"]), f32(inputs["Wb3"])
    Wfc1, Wfc2 = f32(inputs["Wfc1"]), f32(inputs["Wfc2"])

    xr3 = x.reshape(B, CL, IL)  # [b, c, i]
    xT = np.ascontiguousarray(xr3.transpose(2, 1, 0))  # [i, c, b]
    x0T = np.ascontiguousarray(xr3[:, 0, :].T)  # [i, b]

    tnp = ml_dtypes.bfloat16
    w1p_np = np.zeros((128, 2, 128), np.float32)
    w1p_np[0:64, 0, :] = W1[:, :, 0].T
    w1p_np[64:128, 0, :] = W1[:, :, 1].T
    w1p_np[0:64, 1, :] = W1[:, :, 2].T
    w1p_np[64:128, 1, :] = W1[:, :, 2].T

    # fc1 weights -> [8 f, 8 chunks, 128 part, <=9 slabs, 2 ci, 128 f]
    C3_OUT = CH3 * L3
    wg = np.ascontiguousarray(
        Wfc1[:, :C3_OUT].reshape(F1, CH3, L3).transpose(2, 1, 0)
        .reshape(L3, 2, 128, F1).transpose(0, 2, 1, 3)
    )  # [l3, 128, ci, F1]
    wbyp = np.ascontiguousarray(
        Wfc1[:, C3_OUT:].T.reshape(2, 128, F1).transpose(1, 0, 2)
    )  # [128, ci, F1]
    wgf_np = np.zeros((8, NQ, 128, QS, 2, 128), np.float32)
    for f in range(8):
        fsl = slice(f * 128, (f + 1) * 128)
        for q in range(NQ):
            for s in range(NSQ[q]):
                g = OFF[q] + s
                if g < 64:
                    wgf_np[f, q, :, s, :, :] = wg[g, :, :, fsl]
                else:
                    wgf_np[f, q, :, s, :, :] = wbyp[:, :, fsl]

    shared = {
        "wstk": np.ascontiguousarray(
            np.concatenate([Wp[:, :, 1].T, Wp[:, :, 0].T], axis=0)
        ).astype(tnp),
        "w1p": w1p_np.astype(tnp),
        "w2": np.ascontiguousarray(W2.transpose(1, 2, 0)).astype(tnp),
        "w3": np.ascontiguousarray(
            W3.transpose(1, 2, 0).reshape(2, 128, 3, CH3).transpose(1, 0, 2, 3)
        ).astype(tnp),
        "wb1": np.ascontiguousarray(Wb1.T),
        "wb2": np.ascontiguousarray(Wb2.T),
        "wb3": np.ascontiguousarray(Wb3.T),
        "wgf": wgf_np.astype(ml_dtypes.bfloat16),
        "wfc2": np.ascontiguousarray(
            Wfc2.T.reshape(8, 128, OUTC).transpose(1, 0, 2)
        ),
    }

    bias_np = np.zeros((128, NBIAS), np.float32)
    bias_np[:64, 0] = f32(inputs["bp"])
    bias_np[64:, 0] = f32(inputs["bp"])
    bias_np[:, 1] = f32(inputs["b1"])
    b2, b3 = f32(inputs["b2"]), f32(inputs["b3"])
    bias_np[:, 2], bias_np[:, 3] = b2[:128], b2[128:]
    bias_np[:, 4], bias_np[:, 5] = b3[:128], b3[128:]
    bias_np[:64, 6] = f32(inputs["bb1"])
    bias_np[:, 7] = f32(inputs["bb2"])
    bb3 = f32(inputs["bb3"])
    bias_np[:, 8], bias_np[:, 9] = bb3[:128], bb3[128:]
    bias_np[:, 10:18] = f32(inputs["bfc1"]).reshape(8, 128).T
    bias_np[:2, 18] = f32(inputs["bfc2"])
    shared["bias"] = bias_np

    in_maps = []
    for core in range(NCORES):
        sl = slice(core * BC, (core + 1) * BC)
        xc = xT[:, :, sl].reshape(IL, CL, NBLK, BB)
        x0b = x0T[:, sl].reshape(IL, NBLK, BB)
        xs_core = np.empty((NBLK, 128, CL, BB), tnp)
        xs_core[:, :64] = xc.transpose(2, 0, 1, 3)
        xs_core[:, 64:] = x0b.transpose(1, 0, 2)[:, :, None, :]
        m = dict(shared)
        m["xs"] = xs_core
        m["x0s"] = np.ascontiguousarray(x0T[:, sl])
        in_maps.append(m)
    return in_maps


_NC_CACHE = {}


def _get_nc():
    if "nc" not in _NC_CACHE:
        _NC_CACHE["nc"] = build_nc()
    return _NC_CACHE["nc"]


def run(inputs, trace=False):
    from concourse.bass_utils import run_bass_kernel_spmd

    nc = _get_nc()
    in_maps = _prep_inputs(inputs)
    res = run_bass_kernel_spmd(
        nc, in_maps, core_ids=list(range(NCORES)), trace=trace
    )
    outs = [np.asarray(r["out"]) for r in res.results]
    full = np.concatenate([o.T for o in outs], axis=0).astype(np.float32)
    return full, res


def kernel(**inputs) -> np.ndarray:
    full, _ = run(inputs, trace=False)
    return full

